# revision 5
# baseline (speedup 1.0000x reference)
"""Trainium2 Bass kernel for nn_MoELayer_67619965108245 — fp8 DR, pipelined.

Same math/layout as kernel2 (fp8e4 DoubleRow expert stack, fp32 residual,
dT layout, expert-parallel over 8 cores). Differences:

- Software-pipelined SwiGLU: chunk c's Wo matmuls + residual update are
  emitted interleaved into chunk c+1's gate/val j-loop (and the next
  layer's rmsnorm phase), so the PE fills the slots where the j-loop is
  DVE/Act-paced instead of idling through a separate Wo phase.
- RMSNorm partition-reduce runs as fp8 DoubleRow (sq stored fp8).
- Residual update rides the PE: a 256*I fp32r "inject" matmul seeds the
  Wo PSUM accumulator with 256*h, and Act writes (psum/256) straight back
  to h. No Pool adds, no DVE fused-add; the residual tensors live as
  fp32r so the inject/router/projection matmuls run at 1 cycle/row.
"""

import os

os.environ.setdefault("JAX_PLATFORMS", "axon,cpu")

import numpy as np
import ml_dtypes

import concourse.bass as bass
import concourse.tile as tile
import concourse.mybir as mybir
from concourse.bass import ds, ts
from concourse.bass_utils import run_bass_kernel_spmd

B, N, D, E, L = 64, 55, 512, 8, 4
H = 4 * D
NP = 56
T = NP * B
CH = 512
NCH = T // CH    # 7
KD = D // 128    # 4
KD2 = KD // 2
KH = H // 128    # 16
KH2 = KH // 2
NPC = CH // B    # 8
EPS = 1e-8

SW = 4.0
SA = 4.0
SO = 16.0
PS_G = SW * SA
INV_PS_G = 1.0 / PS_G
INV_DELTA = 1.0 / (PS_G * SO)

fp32 = mybir.dt.float32
fp32r = mybir.dt.float32r
bf16 = mybir.dt.bfloat16
fp8 = mybir.dt.float8e4
bf16_np = ml_dtypes.bfloat16
e4m3 = ml_dtypes.float8_e4m3
DR = mybir.MatmulPerfMode.DoubleRow

_MAX_WAITS = 1


def _split_excess_waits(nc, max_waits=_MAX_WAITS):
    for f in nc.m.functions:
        for bb in f.blocks:
            insts = bb.instructions
            i = 0
            while i < len(insts):
                inst = insts[i]
                si = inst.sync_info
                if si is None or si.on_wait is None or len(si.on_wait) <= max_waits:
                    i += 1
                    continue
                waits = list(si.on_wait)
                keep, extra = waits[-max_waits:], waits[:-max_waits]
                nops = []
                for j in range(0, len(extra), max_waits):
                    nops.append(
                        mybir.InstNoOp(
                            name=f"{inst.name}_ws{j}",
                            engine=inst.engine,
                            ins=[],
                            outs=[],
                            sync_info=mybir.SyncInfo(
                                on_wait=extra[j : j + max_waits], on_update=[]
                            ),
                        )
                    )
                inst.sync_info = mybir.SyncInfo(
                    on_wait=keep, on_update=list(si.on_update or [])
                )
                for k, nop in enumerate(nops):
                    insts.insert(i + k, nop)
                i += len(nops) + 1


def build_bass():
    nc = bass.Bass("TRN2", target_bir_lowering=False, debug=False, num_devices=E)

    xT_d = nc.dram_tensor("xT", [KD, 128, T], fp32r, kind="ExternalInput").ap()
    id_d = nc.dram_tensor("idt", [128, 128], fp32r, kind="ExternalInput").ap()
    wg_d = nc.dram_tensor("wg", [L, 128, KD, H], fp8, kind="ExternalInput").ap()
    wv_d = nc.dram_tensor("wv", [L, 128, KD, H], fp8, kind="ExternalInput").ap()
    wo_d = nc.dram_tensor("wo", [L, 128, KH, D], fp8, kind="ExternalInput").ap()
    wr_d = nc.dram_tensor("wr", [128, NP, KD, E], fp32r, kind="ExternalInput").ap()
    br_d = nc.dram_tensor("brt", [1, NP * E], fp32, kind="ExternalInput").ap()
    sel_d = nc.dram_tensor("sel", [E, 1], fp32, kind="ExternalInput").ap()
    wp_d = nc.dram_tensor("wp", [128, KD, 1], fp32r, kind="ExternalInput").ap()
    bp_d = nc.dram_tensor("bps", [1, 1], fp32, kind="ExternalInput").ap()
    u_d = nc.dram_tensor("u", [1, T], fp32, kind="ExternalOutput").ap()

    with tile.TileContext(nc) as tc:
        from contextlib import ExitStack

        with ExitStack() as ctx:
            const = ctx.enter_context(tc.tile_pool(name="const", bufs=1))
            hp = ctx.enter_context(tc.tile_pool(name="hpool", bufs=1))
            wpg = ctx.enter_context(tc.tile_pool(name="wpg", bufs=2))
            wpv = ctx.enter_context(tc.tile_pool(name="wpv", bufs=2))
            wpo = ctx.enter_context(tc.tile_pool(name="wpo", bufs=2))
            nrm = ctx.enter_context(tc.tile_pool(name="nrm", bufs=5))
            sqp = ctx.enter_context(tc.tile_pool(name="sqp", bufs=4))
            gvp = ctx.enter_context(tc.tile_pool(name="gvp", bufs=2))
            silup = ctx.enter_context(tc.tile_pool(name="silup", bufs=6))
            dscp = ctx.enter_context(tc.tile_pool(name="dscp", bufs=3))
            bcp = ctx.enter_context(tc.tile_pool(name="bcp", bufs=4))
            smallp = ctx.enter_context(tc.tile_pool(name="smallp", bufs=4))
            routp = ctx.enter_context(tc.tile_pool(name="routp", bufs=2))
            outp = ctx.enter_context(tc.tile_pool(name="outp", bufs=2))
            pg = ctx.enter_context(tc.tile_pool(name="pg", bufs=2, space="PSUM"))
            pv = ctx.enter_context(tc.tile_pool(name="pv", bufs=2, space="PSUM"))
            pd = ctx.enter_context(tc.tile_pool(name="pd", bufs=2, space="PSUM"))
            pm = ctx.enter_context(tc.tile_pool(name="pm", bufs=2, space="PSUM"))

            # ---- constants ----
            # DR weight APs need k-pair stride %16==0: pad the ones column
            ones_k8 = const.tile([128, 2, 16], fp8, name="ones_k8")
            nc.vector.memset(ones_k8, 1.0)
            ones_m_bf = const.tile([1, 128], bf16, name="ones_m_bf")
            nc.vector.memset(ones_m_bf, 1.0)
            ones_b_f = const.tile([1, B], fp32, name="ones_b_f")
            nc.vector.memset(ones_b_f, 1.0)
            ones_e_f = const.tile([E, 1], fp32, name="ones_e_f")
            nc.vector.memset(ones_e_f, 1.0)
            eps_sb = const.tile([1, 1], fp32, name="eps_sb")
            nc.vector.memset(eps_sb, EPS / (SA * SA))
            sel_sb = const.tile([E, 1], fp32, name="sel_sb")
            nc.sync.dma_start(sel_sb[:], sel_d[:])
            br_sb = const.tile([1, NP * E], fp32, name="br_sb")
            nc.sync.dma_start(br_sb[:], br_d[:])
            wr_sb = const.tile([128, NP, KD, E], fp32r, name="wr_sb")
            nc.sync.dma_start(wr_sb[:], wr_d[:])
            wp_sb = const.tile([128, KD, 1], fp32r, name="wp_sb")
            nc.sync.dma_start(wp_sb[:], wp_d[:])
            id_sb = const.tile([128, 128], fp32r, name="id_sb")
            nc.sync.dma_start(id_sb[:], id_d[:])
            bp_sb = const.tile([1, 1], fp32, name="bp_sb")
            nc.sync.dma_start(bp_sb[:], bp_d[:])
            w_sb = const.tile([1, T], fp32, name="w_sb")

            # ---- residual state, per-chunk DMA ----
            h = []
            for k in range(KD):
                hk = hp.tile([128, T], fp32r, name=f"h{k}", tag=f"h{k}")
                h.append(hk)
            for c in range(NCH):
                cs = ds(c * CH, CH)
                for k in range(KD):
                    nc.sync.dma_start(h[k][:, cs], xT_d[k, :, cs])

            # ---- router ----
            for c in range(NCH):
                cs = ds(c * CH, CH)
                lg = pm.tile([128, CH], fp32, name=f"lg{c}", tag="pm")
                for ni in range(NPC):
                    n = c * NPC + ni
                    off = ni * B
                    for k in range(KD):
                        nc.tensor.matmul(
                            lg[0:E, ds(off, B)],
                            wr_sb[:, n, k, :],
                            h[k][:, ds(n * B, B)],
                            start=(k == 0),
                            stop=False,
                        )
                    nc.tensor.matmul(
                        lg[0:E, ds(off, B)],
                        br_sb[0:1, ds(n * E, E)],
                        ones_b_f[:],
                        start=False,
                        stop=True,
                    )
                expc = routp.tile([E, CH], fp32, name=f"expc{c}", tag="expc")
                nc.scalar.activation(
                    expc[:], lg[0:E, :], mybir.ActivationFunctionType.Exp
                )
                den = pm.tile([128, CH], fp32, name=f"den{c}", tag="pm")
                nc.tensor.matmul(
                    den[0:1, :], ones_e_f[:], expc[:], start=True, stop=True
                )
                num = pd.tile([128, CH], fp32, name=f"num{c}", tag="pd")
                nc.tensor.matmul(
                    num[0:1, :], sel_sb[:], expc[:], start=True, stop=True
                )
                rden = smallp.tile([1, CH], fp32, name=f"rden{c}", tag="rden")
                nc.vector.reciprocal(rden[:], den[0:1, :])
                nc.vector.tensor_mul(w_sb[:, cs], num[0:1, :], rden[:])

            # ---- expert MLP stack, software-pipelined ----
            pending = []  # deferred Wo/resid emitters from the previous chunk

            def drain(k):
                for _ in range(min(k, len(pending))):
                    pending.pop(0)()

            def emit_rms(l, c, nt):
                cs = ds(c * CH, CH)
                sq = sqp.tile([128, KD, CH], fp8, name=f"sq{l}_{c}", tag="sq")
                for k in range(KD):
                    nc.gpsimd.tensor_mul(
                        sq[:, k, :],
                        h[k][:, cs].bitcast(fp32), h[k][:, cs].bitcast(fp32),
                    )
                msq = pm.tile([128, CH], fp32, name=f"ms{l}_{c}", tag="pm")
                for p in range(KD2):
                    nc.tensor.matmul(
                        msq[0:1, :],
                        ones_k8[:, :, 0:1],
                        sq[:, ds(2 * p, 2), :],
                        perf_mode=DR,
                        start=(p == 0),
                        stop=(p == KD2 - 1),
                    )
                std = smallp.tile([1, CH], fp32, name=f"std{l}_{c}", tag="std")
                nc.scalar.activation(
                    std[:], msq[0:1, :], mybir.ActivationFunctionType.Sqrt,
                    bias=eps_sb[:], scale=1.0 / (D * SA * SA),
                )
                rstd = smallp.tile([1, CH], bf16, name=f"rstd{l}_{c}", tag="rstd")
                with nc.allow_low_precision(
                    reason="rstd feeds fp8 normed; bf16 rstd is free precision-wise"
                ):
                    nc.vector.reciprocal(rstd[:], std[:])
                bc = pm.tile([128, CH], fp32, name=f"bc{l}_{c}", tag="pm")
                nc.tensor.matmul(bc[:], ones_m_bf[:], rstd[:], start=True, stop=True)
                bcs = bcp.tile([128, CH], bf16, name=f"bcs{l}_{c}", tag="bcs")
                nc.scalar.activation(
                    bcs[:], bc[:], mybir.ActivationFunctionType.Copy
                )
                for k in range(KD):
                    nc.gpsimd.tensor_mul(
                        nt[:, k, :], h[k][:, cs].bitcast(fp32), bcs[:]
                    )

            def make_owork(l, c, wo_sb, gv):
                """Emitters for chunk c's Wo matmuls + residual update,
                grouped ~3 MMs per slot; final-layer chunks append the
                output projection."""
                cs = ds(c * CH, CH)
                out = []
                state = {}

                def mk_mm(i, p):
                    def f():
                        if p == 0:
                            state[i] = pd.tile(
                                [128, CH], fp32, name=f"d{l}_{c}_{i}", tag="pd"
                            )
                            # seed the accumulator with 256*h on the PE
                            nc.tensor.matmul(
                                state[i][:],
                                id_sb[:],
                                h[i][:, cs],
                                start=True,
                                stop=False,
                                skip_group_check=True,
                            )
                        nc.tensor.matmul(
                            state[i][:],
                            wo_sb[:, ds(2 * p, 2), ts(i, 128)],
                            gv[:, ds(2 * p, 2), :],
                            perf_mode=DR,
                            start=False,
                            stop=(p == KH2 - 1),
                            skip_group_check=True,
                        )
                        if p == KH2 - 1:
                            # h_new = (256*h + 256*delta) / 256, written by Act
                            nc.scalar.activation(
                                h[i][:, cs], state[i][:],
                                mybir.ActivationFunctionType.Copy,
                                scale=INV_DELTA,
                            )
                    return f

                for i in range(KD):
                    for p0 in range(0, KH2, 3):
                        def g(i=i, p0=p0):
                            for p in range(p0, min(p0 + 3, KH2)):
                                mk_mm(i, p)()
                        out.append(g)

                if l == L - 1:
                    def fin():
                        eo = pm.tile([128, CH], fp32, name=f"eo{c}", tag="pm")
                        for k in range(KD):
                            nc.tensor.matmul(
                                eo[0:1, :],
                                wp_sb[:, k, :],
                                h[k][:, cs],
                                start=(k == 0),
                                stop=(k == KD - 1),
                            )
                        eos = outp.tile([1, CH], fp32, name=f"eos{c}", tag="eos")
                        nc.scalar.activation(
                            eos[:], eo[0:1, :],
                            mybir.ActivationFunctionType.Identity,
                            bias=bp_sb[:],
                        )
                        us = outp.tile([1, CH], fp32, name=f"us{c}", tag="us")
                        nc.vector.tensor_mul(us[:], eos[:], w_sb[:, cs])
                        nc.sync.dma_start(u_d[0:1, cs], us[:])
                    out.append(fin)
                return out

            for l in range(L):
                wg_sb = wpg.tile([128, KD, H], fp8, name=f"wg{l}", tag="wg")
                nc.sync.dma_start(wg_sb[:], wg_d[l])
                wv_sb = wpv.tile([128, KD, H], fp8, name=f"wv{l}", tag="wv")
                nc.sync.dma_start(wv_sb[:], wv_d[l])
                wo_sb = wpo.tile([128, KH, D], fp8, name=f"wo{l}", tag="wo")
                nc.sync.dma_start(wo_sb[:], wo_d[l])

                # rmsnorm is interleaved into the j-loop via pending;
                # only chunks 0/1 are emitted eagerly at the layer boundary
                normed = [None] * NCH

                def rms_emitters(l, c):
                    def f():
                        nt = nrm.tile(
                            [128, KD, CH], fp8, name=f"nt{l}_{c}", tag="nt"
                        )
                        normed[c] = nt
                        emit_rms(l, c, nt)
                    return [f]

                for c0 in (0, 1):
                    rms_emitters(l, c0)[0]()

                # SwiGLU j-loop with interleaved rms + previous-chunk Wo work
                for c in range(NCH):
                    if c == 0 and c + 2 < NCH:
                        pending = rms_emitters(l, c + 2) + pending
                    nt = normed[c]
                    gv = gvp.tile([128, KH, CH], fp8, name=f"gv{l}_{c}", tag="gv")
                    for j in range(KH):
                        gps = pg.tile([128, CH], fp32, name=f"g{l}_{c}_{j}", tag="pg")
                        vps = pv.tile([128, CH], fp32, name=f"v{l}_{c}_{j}", tag="pv")
                        for p in range(KD2):
                            nc.tensor.matmul(
                                gps[:],
                                wg_sb[:, ds(2 * p, 2), ts(j, 128)],
                                nt[:, ds(2 * p, 2), :],
                                perf_mode=DR,
                                start=(p == 0),
                                stop=(p == KD2 - 1),
                            )
                        for p in range(KD2):
                            nc.tensor.matmul(
                                vps[:],
                                wv_sb[:, ds(2 * p, 2), ts(j, 128)],
                                nt[:, ds(2 * p, 2), :],
                                perf_mode=DR,
                                start=(p == 0),
                                stop=(p == KD2 - 1),
                            )
                        sil = silup.tile(
                            [128, CH], bf16, name=f"sl{l}_{c}_{j}", tag="sil"
                        )
                        nc.scalar.activation(
                            sil[:], gps[:], mybir.ActivationFunctionType.Silu,
                            scale=INV_PS_G,
                        )
                        nc.vector.tensor_mul(gv[:, j, :], sil[:], vps[:])
                        drain(1)
                    drain(len(pending))  # safety: should already be empty
                    pending = (
                        rms_emitters(l, c + 3) if c + 3 < NCH else []
                    ) + make_owork(l, c, wo_sb, gv)

            drain(len(pending))

    _split_excess_waits(nc)
    return nc


_CACHE = {}


def _get_nc():
    if "nc" not in _CACHE:
        _CACHE["nc"] = build_bass()
    return _CACHE["nc"]


def _prep_inputs(x, scale, Wg, Wv, Wo, Wp, bp, Wr, br):
    x = np.asarray(x, np.float32)
    scale = np.asarray(scale, np.float32)
    Wg = np.asarray(Wg, np.float32)
    Wv = np.asarray(Wv, np.float32)
    Wo = np.asarray(Wo, np.float32)
    Wp = np.asarray(Wp, np.float32)
    bp = np.asarray(bp, np.float32)
    Wr = np.asarray(Wr, np.float32)
    br = np.asarray(br, np.float32)

    xt = np.zeros((D, NP, B), np.float32)
    xt[:, :N, :] = x.transpose(2, 1, 0)
    xT = np.ascontiguousarray(xt.reshape(KD, 128, T))

    wr_full = np.zeros((NP, E, D), np.float32)
    wr_full[:N] = Wr
    wr_prep = np.ascontiguousarray(
        wr_full.transpose(2, 0, 1).reshape(KD, 128, NP, E).transpose(1, 2, 0, 3)
    )
    br_full = np.zeros((NP, E), np.float32)
    br_full[:N] = br
    br_prep = np.ascontiguousarray(br_full.reshape(1, NP * E))

    wg_eff = Wg * scale[:, :, :, None]
    wv_eff = Wv * scale[:, :, :, None]

    ident = np.ascontiguousarray(np.eye(128, dtype=np.float32) * (PS_G * SO))
    in_maps = []
    for e in range(E):
        wg_p = np.ascontiguousarray(
            (wg_eff[:, e] * SW).reshape(L, KD, 128, H).transpose(0, 2, 1, 3)
        ).astype(e4m3)
        wv_p = np.ascontiguousarray(
            (wv_eff[:, e] * SW).reshape(L, KD, 128, H).transpose(0, 2, 1, 3)
        ).astype(e4m3)
        wo_p = np.ascontiguousarray(
            (Wo[:, e] * SO).reshape(L, KH, 128, D).transpose(0, 2, 1, 3)
        ).astype(e4m3)
        wp_p = np.ascontiguousarray(
            Wp[e].reshape(KD, 128, 1).transpose(1, 0, 2)
        )
        sel = np.zeros((E, 1), np.float32)
        sel[e, 0] = 1.0
        in_maps.append(
            {
                "xT": xT,
                "idt": ident,
                "wg": wg_p,
                "wv": wv_p,
                "wo": wo_p,
                "wr": wr_prep,
                "brt": br_prep,
                "sel": sel,
                "wp": wp_p,
                "bps": np.array([[bp[e]]], np.float32),
            }
        )
    return in_maps


def _combine(results):
    u = np.zeros(T, np.float64)
    for r in results:
        u += r["u"].reshape(T).astype(np.float64)
    return np.ascontiguousarray(u.reshape(NP, B)[:N, :].T).astype(np.float32)


def kernel(x, scale, Wg, Wv, Wo, Wp, bp, Wr, br):
    nc = _get_nc()
    in_maps = _prep_inputs(x, scale, Wg, Wv, Wo, Wp, bp, Wr, br)
    res = run_bass_kernel_spmd(nc, in_maps, list(range(E)))
    return _combine(res.results)
